# revision 1
# baseline (speedup 1.0000x reference)
# Viterbi (CRF max-plus decode) on 8 Trainium2 NeuronCores.
#
# Contract: kernel(**inputs) takes the FULL inputs of reference.setup_inputs()
# and returns the FULL output (paths [S,B] int32, best_scores [B] f32).
#
# Strategy (bitwise-exact vs the fp32 reference):
#  - Shard batch B=256 across 8 cores (32 per core); the S=512 scan is local.
#  - Per core, per step s: sc[b,prev,cur] = fl(v[b,prev] + trans[prev,cur]) is
#    built as one DVE tensor_tensor add (partition=(cc,b) packing, free=
#    (ci=32, prev=128), cur = cc*32+ci) with a step-0 broadcast AP on v —
#    partially offloaded to GPSIMD; best = segmented reduce_max on DVE
#    (bit-exact fp32). v' = best + feats[s] is assembled across partitions by
#    one-hot fp32 PE matmuls (selection + accumulate = single-rounded adds,
#    exact), then copied PSUM->SBUF by the scalar engine.
#  - Each v_s [32,128] streams to HBM. The host recomputes backpointers
#    (argmax) from the streamed v_s — identical fp32 values => identical
#    argmax as the reference — and runs the O(S*B) backtrack in numpy.
import numpy as np
from contextlib import ExitStack

S, B, T = 512, 256, 128
NCORES = 8
BL = B // NCORES          # 32 batch rows per core
CC, CI = 4, 32            # partition packing: p = cc*32 + b ; cur = cc*32 + ci
GP_CI = 20                # ci-blocks built on GPSIMD (the rest on DVE)

_cache = {}


def _build_module(nstep):
    import concourse.tile as tile
    from concourse import bacc, mybir

    FD = mybir.dt.float32
    nc = bacc.Bacc("TRN2", target_bir_lowering=False, debug=False,
                   num_devices=NCORES)
    d_tb = nc.dram_tensor("tb", [128, CI * T], FD, kind="ExternalInput")
    d_v0 = nc.dram_tensor("v0", [128, T], FD, kind="ExternalInput")
    d_f = nc.dram_tensor("f", [nstep, BL, T], FD, kind="ExternalInput")
    d_lr = nc.dram_tensor("lr", [128, CC * 128], FD, kind="ExternalInput")
    d_lf = nc.dram_tensor("lf", [BL, 128], FD, kind="ExternalInput")
    d_vh = nc.dram_tensor("vh", [nstep, BL, T], FD, kind="ExternalOutput")

    with tile.TileContext(nc) as tc:
        with ExitStack() as ctx:
            const = ctx.enter_context(tc.tile_pool(name="const", bufs=1))
            sc_pool = ctx.enter_context(tc.tile_pool(name="sc", bufs=2))
            vr_pool = ctx.enter_context(tc.tile_pool(name="vr", bufs=2))
            best_pool = ctx.enter_context(tc.tile_pool(name="best", bufs=2))
            f_pool = ctx.enter_context(tc.tile_pool(name="f", bufs=4))
            psum = ctx.enter_context(tc.tile_pool(name="ps", bufs=2, space="PSUM"))

            s_tb = const.tile([128, CI * T], FD)
            s_lr = const.tile([128, CC * 128], FD)
            s_lf = const.tile([BL, 128], FD)
            nc.sync.dma_start(s_tb[:], d_tb[:])
            nc.sync.dma_start(s_lr[:], d_lr[:])
            nc.sync.dma_start(s_lf[:], d_lf[:])
            vr = vr_pool.tile([128, T], FD)
            nc.sync.dma_start(vr[:], d_v0[:])

            tb3 = s_tb[:].rearrange("p (c q) -> p c q", c=CI)
            for s in range(nstep):
                f_t = f_pool.tile([BL, T], FD)
                nc.sync.dma_start(f_t[:], d_f[s])
                sc = sc_pool.tile([128, CI * T], FD)
                sc3 = sc[:].rearrange("p (c q) -> p c q", c=CI)
                best = best_pool.tile([128, CI], FD)
                # SC = TB + v_bcast, split GPSIMD/DVE by ci-blocks; reduce per
                # region so the DVE reduce of a region follows its own add.
                if GP_CI > 0:
                    h = GP_CI // 2
                    for lo, hi in ((0, h), (h, GP_CI)):
                        nc.gpsimd.tensor_add(
                            sc3[:, lo:hi], tb3[:, lo:hi],
                            vr[:, None, :].broadcast_to([128, hi - lo, T]))
                        nc.vector.tensor_reduce(
                            best[:, None, lo:hi], sc3[:, lo:hi],
                            axis=mybir.AxisListType.X, op=mybir.AluOpType.max)
                nc.vector.tensor_add(
                    sc3[:, GP_CI:CI], tb3[:, GP_CI:CI],
                    vr[:, None, :].broadcast_to([128, CI - GP_CI, T]))
                nc.vector.tensor_reduce(
                    best[:, None, GP_CI:CI], sc3[:, GP_CI:CI],
                    axis=mybir.AxisListType.X, op=mybir.AluOpType.max)
                # v' = remap(best) + feats : one-hot fp32 matmuls, exact.
                p_vn = psum.tile([128, T], FD)
                for cc in range(CC):
                    nc.tensor.matmul(p_vn[:, cc * 32:(cc + 1) * 32],
                                     s_lr[:, cc * 128:(cc + 1) * 128],
                                     best[:], start=True, stop=False)
                    nc.tensor.matmul(p_vn[:, cc * 32:(cc + 1) * 32], s_lf[:],
                                     f_t[:, cc * 32:(cc + 1) * 32],
                                     start=False, stop=True)
                vr = vr_pool.tile([128, T], FD)
                nc.scalar.copy(vr[:], p_vn[:])
                nc.sync.dma_start(d_vh[s], vr[0:BL, :])
    nc.compile()
    return nc


def _static_tiles(trans):
    # TB[p=(cc,b), ci*T+prev] = trans[prev, cc*32+ci]  (b-replicated)
    tbc = np.ascontiguousarray(trans.T)              # [cur, prev]
    TB = np.empty((128, CI * T), np.float32)
    for cc in range(CC):
        blk = tbc[cc * 32:(cc + 1) * 32].reshape(CI * T)
        TB[cc * 32:(cc + 1) * 32] = blk
    # remap lhsT: out[(cco,b), cc*32+ci] = best[(cc,b), ci]
    lr = np.zeros((CC, 128, 128), np.float32)
    for cc in range(CC):
        for b in range(BL):
            for cco in range(CC):
                lr[cc, cc * 32 + b, cco * 32 + b] = 1.0
    LR = lr.transpose(1, 0, 2).reshape(128, CC * 128)
    # feats lhsT: out[(cco,b), cur] += f[b, cur]
    LF = np.zeros((BL, 128), np.float32)
    for b in range(BL):
        for cco in range(CC):
            LF[b, cco * 32 + b] = 1.0
    return TB, LR, LF


def kernel(feats, transitions, start_transitions, end_transitions):
    from concourse.bass_utils import run_bass_kernel_spmd

    feats = np.asarray(feats, dtype=np.float32)
    trans = np.asarray(transitions, dtype=np.float32)
    start_t = np.asarray(start_transitions, dtype=np.float32)
    end_t = np.asarray(end_transitions, dtype=np.float32)
    nstep = S - 1

    if "nc" not in _cache:
        _cache["nc"] = _build_module(nstep)
    nc = _cache["nc"]

    TB, LR, LF = _static_tiles(trans)
    v0 = feats[0] + start_t                      # [B, T] fp32, same fl as ref
    in_maps = []
    for c in range(NCORES):
        sl = slice(c * BL, (c + 1) * BL)
        in_maps.append(dict(
            tb=TB, lr=LR, lf=LF,
            v0=np.tile(v0[sl], (CC, 1)),         # [(cc,b), prev] replicated
            f=np.ascontiguousarray(feats[1:, sl, :]),
        ))
    res = run_bass_kernel_spmd(nc, in_maps, list(range(NCORES)))

    # vhist[s] = v_s for s = 1..S-1 ; v_0 known on host.
    vh = np.empty((S, B, T), np.float32)
    vh[0] = v0
    for c in range(NCORES):
        vh[1:, c * BL:(c + 1) * BL, :] = res.results[c]["vh"]

    # Final scoring + host backtrack (exact recompute of argmax decisions).
    v_final = vh[S - 1] + end_t                  # [B, T]
    best_last = np.argmax(v_final, axis=-1).astype(np.int32)    # [B]
    best_scores = np.max(v_final, axis=-1)                       # [B]

    trans_T = np.ascontiguousarray(trans.T)      # [cur, prev]
    paths = np.empty((S, B), np.int32)
    paths[S - 1] = best_last
    nxt = best_last
    for s in range(S - 1, 0, -1):
        # bp[b] = argmax_prev( v_{s-1}[b, prev] + trans[prev, nxt[b]] )
        scores = vh[s - 1] + trans_T[nxt]        # [B, T] fp32, identical fl
        nxt = np.argmax(scores, axis=1).astype(np.int32)
        paths[s - 1] = nxt
    return paths, best_scores
